# revision 29
# baseline (speedup 1.0000x reference)
# Multi-headed attention + residual + LayerNorm, head-parallel over 8 cores.
#
# Sharding: core c handles batch b = c // 4 and head block r = c % 4
# (heads 4r..4r+3, two pairs). Q/K/V projections are computed locally for
# the core's own heads over ALL 2048 tokens (no K/V communication).
# After attention, per-head outputs (UNnormalized, with a Z row appended)
# are exchanged with one AllToAll per token-quarter: each rank stages its
# [2 pair, 2 head, 65, 128] block per destination rank and receives
# exactly its own 128-token slice of every head. 1/Z normalization happens
# post-gather in o_block (keeps the reciprocal chain out of the main loop),
# followed by O-projection + residual + LayerNorm locally.
#
# All projection matmuls and the PV matmul run fp8 DoubleRow (256-deep
# contraction per instruction, 2x PE throughput). exp(s/8) is split
# between ScalarE (ACT Exp -> fp8) and VectorE using the Schraudolph int
# trick in fp8e4 bit space:
#   fp8_bits(2^y) ~= round(y*8 + 56 - C) for y = s * 0.125 * log2(e)
# computed as one tensor_scalar (mult+add, f32 PSUM -> int8 SBUF), the
# int8 tile bitcast to fp8e4 feeds the PV DoubleRow matmul directly.
import math

import numpy as np
import ml_dtypes

BF16 = ml_dtypes.bfloat16
B, S, DM = 2, 2048, 1024
NH, DH = 16, 64
P = 128
CC = DM // P            # 8 contraction chunks of 128
RH = 4                  # heads per core
NQ = 4                  # token quarters
QT = S // NQ            # 512 tokens per quarter
TB = P                  # 128-token output block
KT_CH = S // P          # 16 key chunks of 128
EPS = 1e-6

# odd kc-groups run exp on VectorE via the int trick (8/16 split keeps
# ScalarE and VectorE balanced)
DVE_KC = tuple(g for g in range(16) if g % 2 == 1)
# fp8e4m3-bit-space constants: bits(2^y) ~= y*8 + 56 - 0.344
A8_CONST = 0.125 * 8.0 / math.log(2.0)
B8_CONST = 56.0 - 0.344

_NC = None


def _build_nc():
    import concourse.bass as bass
    import concourse.mybir as mybir
    import concourse.tile as tile
    from concourse import bacc

    f32 = mybir.dt.float32
    bf16 = mybir.dt.bfloat16
    i8 = mybir.dt.int8
    fp8 = mybir.dt.float8e4
    Alu = mybir.AluOpType
    Act = mybir.ActivationFunctionType

    nc = bacc.Bacc(num_devices=8)
    DR = mybir.MatmulPerfMode.DoubleRow

    xqT_d = nc.dram_tensor("xqT", [DM, S], fp8, kind="ExternalInput")
    xkT_d = nc.dram_tensor("xkT", [DM, S], fp8, kind="ExternalInput")
    xvT_d = nc.dram_tensor("xvT", [DM, S], fp8, kind="ExternalInput")
    wqT_d = nc.dram_tensor("wqT", [DM, 2 * P], fp8, kind="ExternalInput")
    wkT_d = nc.dram_tensor("wkT", [DM, 2 * P], fp8, kind="ExternalInput")
    wvT_d = nc.dram_tensor("wvT", [DM, 2 * P], fp8, kind="ExternalInput")
    woT_d = nc.dram_tensor("woT", [DM, DM], fp8, kind="ExternalInput")
    bqp_d = nc.dram_tensor("bqp", [P, 2], f32, kind="ExternalInput")
    bkp_d = nc.dram_tensor("bkp", [P, 2], f32, kind="ExternalInput")
    vecs_d = nc.dram_tensor("vecs", [2, DM], bf16, kind="ExternalInput")
    onehot_d = nc.dram_tensor("onehot", [2, P], bf16, kind="ExternalInput")
    resid_d = nc.dram_tensor("resid", [NQ * TB, DM], f32, kind="ExternalInput")
    colofs_d = nc.dram_tensor("colofs", [1, 1], mybir.dt.uint32, kind="ExternalInput")
    out_d = nc.dram_tensor("out", [NQ * TB, DM], f32, kind="ExternalOutput")
    # per-quarter AllGather staging: own [pair, 128 xu rows (hb-major) +
    # 2 Z rows, 512 tok] -> gathered [src rank, pair, 130, 512]. Z rows at
    # the END keep the xu block contiguous for single-DMA assembly reads.
    xs_d = [
        nc.dram_tensor(f"xs{q}", [2, 2 * DH + 2, QT], fp8, kind="Internal")
        for q in range(NQ)
    ]
    xg_d = [
        nc.dram_tensor(f"xg{q}", [4, 2, 2 * DH + 2, QT], fp8, kind="Internal")
        for q in range(NQ)
    ]
    # quarter 3 uses one AG per pair so the pair-0 gather overlaps the
    # pair-1 attention unit and only a half-size AG remains in the tail
    xs3_d = [
        nc.dram_tensor(f"xs3p{p}", [2 * DH + 2, QT], fp8, kind="Internal")
        for p in range(2)
    ]
    xg3_d = [
        nc.dram_tensor(f"xg3p{p}", [4, 2 * DH + 2, QT], fp8, kind="Internal")
        for p in range(2)
    ]
    # tiny warmup collective (absorbs the ~60us first-collective barrier)
    cwu_in_d = nc.dram_tensor("cwui", [2, 16], bf16, kind="Internal")
    cwu_out_d = nc.dram_tensor("cwuo", [8, 16], bf16, kind="Internal")

    groups = [[0, 1, 2, 3], [4, 5, 6, 7]]

    with tile.TileContext(nc) as tc:
        from concourse.hw_specs import get_activation_tables

        tables = get_activation_tables(nc.m.arch)
        set_id = list(tables.keys()).index("natural_log_exp_and_others")
        nc.scalar.add_instruction(
            mybir.InstLoadActFuncSet(
                name=nc.get_next_instruction_name(),
                act_func_set_id=set_id,
                ins=[],
                outs=[],
            )
        )
        nc.gpsimd.collective_compute(
            "AllGather",
            mybir.AluOpType.bypass,
            replica_groups=groups,
            ins=[cwu_in_d[:, :]],
            outs=[cwu_out_d[:, :]],
        )
        with (
            tc.tile_pool(name="const", bufs=1) as const,
            tc.tile_pool(name="wpool", bufs=1) as wpool,
            tc.tile_pool(name="acts", bufs=1) as acts,
            tc.tile_pool(name="xin", bufs=4) as xin,
            tc.tile_pool(name="epool", bufs=3) as epool,
            tc.tile_pool(name="xupool", bufs=3) as xupool,
            tc.tile_pool(name="zpool", bufs=3) as zpool,
            tc.tile_pool(name="opool", bufs=2) as opool,
            tc.tile_pool(name="small", bufs=4) as small,
            tc.tile_pool(name="pmain", bufs=2, space="PSUM") as pmain,
            tc.tile_pool(name="ppv", bufs=2, space="PSUM") as ppv,
            tc.tile_pool(name="pproj", bufs=2, space="PSUM") as pproj,
        ):
            # ---------------- persistent activations ----------------
            kT = acts.tile([P, 2, S], bf16, name="kT")
            qT = acts.tile([P, 2, S], bf16, name="qT")
            # 128 cols per (chunk, head): 64 V rows, one 1/16-row (Z), 63 zero
            # pad (DoubleRow LDWEIGHTS requires a full 128-column stationary).
            # The 1/16 Z-row makes a single x(1/16) evacuation produce
            # xup/16 and Z/256 uniformly.
            vsb = acts.tile([P, KT_CH, RH, 2 * DH], fp8, name="vsb")
            xq = acts.tile([P, CC, S], fp8, name="xq")

            # ---------------- K-projection inputs first ----------------
            wk = wpool.tile([P, CC, 2 * P], fp8, name="wk")
            nc.sync.dma_start(
                out=wk, in_=wkT_d[:, :].rearrange("(c p) n -> p c n", p=P)
            )
            xkr = xkT_d[:, :].rearrange("(c p) t -> p c t", p=P)
            xk_g = []
            for h_, eng in zip(range(2), (nc.scalar, nc.gpsimd)):
                t_ = xin.tile([P, 4, S], fp8, tag="xin", name=f"xk{h_}")
                eng.dma_start(out=t_, in_=xkr[:, 4 * h_ : 4 * h_ + 4, :])
                xk_g.append(t_)
            # contraction-chunk PAIRS [P, 2, S] for fp8 DoubleRow matmuls
            xk_pr = [
                xk_g[j // 2][:, 2 * (j % 2) : 2 * (j % 2) + 2, :]
                for j in range(CC // 2)
            ]
            bkp = const.tile([P, 2], f32, name="bkp_sb")
            nc.sync.dma_start(out=bkp, in_=bkp_d[:, :])
            nc.vector.memset(vsb[:, :, :, DH : DH + 1], 1.0 / 16.0)
            nc.vector.memset(vsb[:, :, :, DH + 1 :], 0.0)

            def kproj_block(p_, tb):
                ps = pproj.tile([P, 512], f32, tag="pj", name="ps_k")
                for j in range(CC // 2):
                    nc.tensor.matmul(
                        ps,
                        wk[:, 2 * j : 2 * j + 2, p_ * P : (p_ + 1) * P],
                        xk_pr[j][:, :, tb * 512 : (tb + 1) * 512],
                        start=(j == 0),
                        stop=(j == CC // 2 - 1),
                        perf_mode=DR,
                    )
                nc.vector.tensor_scalar(
                    out=kT[:, p_, tb * 512 : (tb + 1) * 512],
                    in0=ps,
                    scalar1=1.0 / 16.0,
                    scalar2=bkp[:, p_ : p_ + 1],
                    op0=Alu.mult,
                    op1=Alu.add,
                )

            # both K pairs up front (frees the xk slots for xv)
            for p_ in range(2):
                for tb in range(4):
                    kproj_block(p_, tb)

            # Q-projection inputs for quarter 0
            wq = wpool.tile([P, CC, 2 * P], fp8, name="wq")
            nc.sync.dma_start(
                out=wq, in_=wqT_d[:, :].rearrange("(c p) n -> p c n", p=P)
            )
            xqr = xqT_d[:, :].rearrange("(c p) t -> p c t", p=P)
            for h_ in range(2):
                nc.sync.dma_start(
                    out=xq[:, 4 * h_ : 4 * h_ + 4, 0:QT],
                    in_=xqr[:, 4 * h_ : 4 * h_ + 4, 0:QT],
                )
            bqp = const.tile([P, 2], f32, name="bqp_sb")
            nc.sync.dma_start(out=bqp, in_=bqp_d[:, :])
            ohh = const.tile([2, P], bf16, name="ohh")
            nc.sync.dma_start(out=ohh, in_=onehot_d[:, :])
            # V inputs + remaining weights
            wv = wpool.tile([P, CC, 2 * P], fp8, name="wv")
            nc.sync.dma_start(
                out=wv, in_=wvT_d[:, :].rearrange("(c p) n -> p c n", p=P)
            )
            xvr = xvT_d[:, :].rearrange("(c p) t -> p c t", p=P)
            xv_g = []
            for h_, eng in zip(range(2), (nc.scalar, nc.gpsimd)):
                t_ = xin.tile([P, 4, S], fp8, tag="xin", name=f"xv{h_}")
                # halves so early v_chunks don't wait for the whole tile
                eng.dma_start(
                    out=t_[:, :, 0 : S // 2],
                    in_=xvr[:, 4 * h_ : 4 * h_ + 4, 0 : S // 2],
                )
                eng.dma_start(
                    out=t_[:, :, S // 2 :],
                    in_=xvr[:, 4 * h_ : 4 * h_ + 4, S // 2 :],
                )
                xv_g.append(t_)
            xv_pr = [
                xv_g[j // 2][:, 2 * (j % 2) : 2 * (j % 2) + 2, :]
                for j in range(CC // 2)
            ]
            wo = wpool.tile([P, CC, DM], fp8, name="wo")
            nc.sync.dma_start(
                out=wo, in_=woT_d[:, :].rearrange("(c p) n -> p c n", p=P)
            )

            vrep = const.tile([P, 2, DM], bf16, name="vrep")
            vecs_ap = vecs_d[:, :]
            vecs_bc = bass.AP(
                tensor=vecs_ap.tensor,
                offset=vecs_ap.offset,
                ap=[[0, P]] + [list(p) for p in vecs_ap.ap],
            )
            nc.gpsimd.dma_start(out=vrep, in_=vecs_bc)


            def v_chunk(tc_):
                # V for token chunk tc_: [128 tok, 4 heads x 64]. The V bias
                # is folded into the residual on the host (softmax weights
                # sum to 1), so this is a pure scale+cast.
                psv = pproj.tile([P, 2 * P], f32, tag="pj", name="ps_v")
                for j in range(CC // 2):
                    nc.tensor.matmul(
                        psv,
                        xv_pr[j][:, :, tc_ * P : (tc_ + 1) * P],
                        wv[:, 2 * j : 2 * j + 2, :],
                        start=(j == 0),
                        stop=(j == CC // 2 - 1),
                        perf_mode=DR,
                    )
                nc.vector.tensor_scalar(
                    out=vsb[:, tc_, :, 0:DH],
                    in0=psv.rearrange("p (h d) -> p h d", d=DH),
                    scalar1=1.0 / 16.0,
                    scalar2=None,
                    op0=Alu.mult,
                )

            def q_proj(p_, q):
                ps = pproj.tile([P, 512], f32, tag="pj", name="ps_q")
                for j in range(CC // 2):
                    nc.tensor.matmul(
                        ps,
                        wq[:, 2 * j : 2 * j + 2, p_ * P : (p_ + 1) * P],
                        xq[:, 2 * j : 2 * j + 2, q * QT : (q + 1) * QT],
                        start=(j == 0),
                        stop=(j == CC // 2 - 1),
                        perf_mode=DR,
                    )
                nc.vector.tensor_scalar(
                    out=qT[:, p_, q * QT : (q + 1) * QT],
                    in0=ps,
                    scalar1=1.0 / 16.0,
                    scalar2=bqp[:, p_ : p_ + 1],
                    op0=Alu.mult,
                    op1=Alu.add,
                )

            def attn_unit(p_, q, feed=None):
                # attention for head pair p_, token quarter q.
                # feed: list of closures to interleave (V chunks, o_block
                # stages), one per kc-PAIR step (8 slots).
                # The PV matmul is software-pipelined ONE gg step behind the
                # score matmuls so the in-order PE queue never waits on the
                # exp that was just issued. exp runs on ScalarE for even gg
                # and VectorE for odd gg (whole et tile has one writer
                # engine, and the engines leapfrog across steps).
                pv = [
                    ppv.tile([P, 512], f32, tag="pv", name=f"pv{h}")
                    for h in range(2)
                ]
                NG = KT_CH // 2

                def do_pv(et_, gg_):
                    for hb in range(2):
                        hl = p_ * 2 + hb
                        nc.tensor.matmul(
                            pv[hb][:, :],
                            vsb[:, 2 * gg_ : 2 * gg_ + 2, hl, :],
                            et_[:, :, hb, :],
                            start=(gg_ == 0),
                            stop=(gg_ == NG - 1),
                            perf_mode=DR,
                        )

                prev = None
                for gg in range(NG):
                    if feed:
                        feed.pop(0)()
                    # et packs two key chunks (gsub) so the PV matmul can run
                    # fp8 DoubleRow with a 256-deep contraction
                    et = epool.tile([P, 2, 2, 512], fp8, tag="et", name="et")
                    for gs in range(2):
                        g = 2 * gg + gs
                        ps = pmain.tile([P, 2, 512], f32, tag="ps", name="ps_s")
                        for hb in range(2):
                            rb = hb * DH
                            nc.tensor.matmul(
                                ps[:, hb, :],
                                kT[rb : rb + DH, p_, g * P : (g + 1) * P],
                                qT[rb : rb + DH, p_, q * QT : (q + 1) * QT],
                                start=True,
                                stop=True,
                            )
                        if gs == 1:
                            nc.vector.tensor_scalar(
                                out=et[:, gs].bitcast(i8),
                                in0=ps,
                                scalar1=A8_CONST,
                                scalar2=B8_CONST,
                                op0=Alu.mult,
                                op1=Alu.add,
                            )
                        else:
                            nc.scalar.activation(
                                out=et[:, gs], in_=ps, func=Act.Exp, scale=0.125
                            )
                    if prev is not None:
                        do_pv(*prev)
                    prev = (et, gg)
                do_pv(*prev)
                # evacuate rows 0..64 (xup/16 and Z/256) straight to fp8 and
                # stage for the AllToAll: dest layout [rank, pair, hb, row, tok]
                stg = xupool.tile([DH + 1, 2, QT], fp8, tag="stg", name="stg")
                for hb in range(2):
                    nc.scalar.activation(
                        out=stg[:, hb, :],
                        in_=pv[hb][0 : DH + 1, :],
                        func=Act.Copy,
                        scale=1.0 / 16.0,
                    )
                if q < NQ - 1:
                    xu_dst = xs_d[q][p_, 0 : 2 * DH, :]
                    z_dst = xs_d[q][p_, 2 * DH : 2 * DH + 2, :]
                else:
                    xu_dst = xs3_d[p_][0 : 2 * DH, :]
                    z_dst = xs3_d[p_][2 * DH : 2 * DH + 2, :]
                nc.sync.dma_start(
                    out=xu_dst.rearrange("(hb r) t -> r hb t", hb=2),
                    in_=stg[0:DH, :, :],
                )
                nc.sync.dma_start(
                    out=z_dst,
                    in_=stg[DH : DH + 1, :, :],
                )

            def launch_ag(q):
                nc.gpsimd.collective_compute(
                    "AllGather",
                    mybir.AluOpType.bypass,
                    replica_groups=groups,
                    ins=[xs_d[q][:, :, :]],
                    outs=[xg_d[q][:, :, :, :]],
                )

            def launch_ag3(p_):
                nc.gpsimd.collective_compute(
                    "AllGather",
                    mybir.AluOpType.bypass,
                    replica_groups=groups,
                    ins=[xs3_d[p_][:, :]],
                    outs=[xg3_d[p_][:, :, :]],
                )

            def o_dma(q, st, pairs=(0, 1)):
                # own 128-token block: assemble gathered xu + Z rows.
                # Pure DMA — its only wait is on AllGather(q) completion,
                # which blocks nothing but the dynamic DMA queue.
                def gsrc(p_):
                    if q < NQ - 1:
                        return xg_d[q][:, p_]
                    return xg3_d[p_][:]

                if "xo" not in st:
                    st["xo"] = opool.tile([P, CC, TB], fp8, tag="xo", name="xo")
                    st["zos"] = [None, None]
                    rs = opool.tile([P, DM], f32, tag="rs", name="rs")
                    nc.scalar.dma_start(
                        out=rs, in_=resid_d[q * TB : (q + 1) * TB, :]
                    )
                    st["rs"] = rs
                xo = st["xo"]
                for p_ in pairs:
                    nc.sync.dma_start(
                        out=xo[:, p_ : CC : 2, :],
                        in_=gsrc(p_)[
                            :, 0 : 2 * DH, bass.ds(colsv, TB)
                        ].rearrange("s d t -> d s t"),
                    )
                for p_ in pairs:
                    zo = zpool.tile([2, 4, TB], fp8, tag="zo", name="zo")
                    nc.sync.dma_start(
                        out=zo,
                        in_=gsrc(p_)[
                            :, 2 * DH : 2 * DH + 2, bass.ds(colsv, TB)
                        ].rearrange("s h t -> h s t"),
                    )
                    st["zos"][p_] = zo

            def o_dve(q, st, pairs=(0, 1)):
                # 1/Z reciprocal chain (VectorE) — woven a few slots after
                # o_dma so the DVE queue never head-of-line blocks on the
                # gather.
                if "zobs" not in st:
                    st["zobs"] = [None, None]
                for p_ in pairs:
                    zo = st["zos"][p_]
                    zof = zpool.tile([2, 4, TB], f32, tag="zof", name="zof")
                    nc.vector.tensor_copy(out=zof, in_=zo)
                    zoi = zpool.tile([2, 4, TB], f32, tag="zoi", name="zoi")
                    zos = zpool.tile([2, 4, TB], f32, tag="zos", name="zos")
                    nc.vector.reciprocal_approx_accurate(
                        zoi.rearrange("a r t -> a (r t)"),
                        zof.rearrange("a r t -> a (r t)"),
                        scratch=zos.rearrange("a r t -> a (r t)"),
                    )
                    zob = zpool.tile([2, 4, TB], bf16, tag="zob", name="zob")
                    nc.vector.tensor_copy(out=zob, in_=zoi)
                    st["zobs"][p_] = zob

            def o_zz(q, st, pairs=(0, 1)):
                # 1/Z spread + multiply per pair
                xo = st["xo"]
                for p_ in pairs:
                    zob = st["zobs"][p_]
                    zz = pproj.tile([P, 4 * TB], f32, tag="pj", name="zz")
                    nc.tensor.matmul(
                        zz,
                        ohh[:, :],
                        zob.rearrange("h r t -> h (r t)"),
                        start=True,
                        stop=True,
                    )
                    nc.vector.tensor_tensor(
                        out=xo[:, p_ : CC : 2, :],
                        in0=xo[:, p_ : CC : 2, :],
                        in1=zz.rearrange("p (r t) -> p r t", r=4),
                        op=Alu.mult,
                    )

            def o_fin(q, st):
                # O-projection + residual + LayerNorm + store
                xo, rs = st["xo"], st["rs"]
                y = opool.tile([P, DM], f32, tag="y", name="y")
                s1h = small.tile([P, 2], f32, tag="s1h", name="s1h")
                for half in range(2):
                    pso = pproj.tile([P, 512], f32, tag="pj", name="ps_o")
                    for j in range(CC // 2):
                        nc.tensor.matmul(
                            pso,
                            xo[:, 2 * j : 2 * j + 2, :],
                            wo[:, 2 * j : 2 * j + 2, half * 512 : (half + 1) * 512],
                            start=(j == 0),
                            stop=(j == CC // 2 - 1),
                            perf_mode=DR,
                        )
                    nc.vector.scalar_tensor_tensor(
                        out=y[:, half * 512 : (half + 1) * 512],
                        in0=pso,
                        scalar=1.0 / 256.0,
                        in1=rs[:, half * 512 : (half + 1) * 512],
                        op0=Alu.mult,
                        op1=Alu.add,
                        accum_out=s1h[:, half : half + 1],
                    )
                s1 = small.tile([P, 1], f32, tag="s1", name="s1")
                nc.vector.tensor_tensor(
                    out=s1,
                    in0=s1h[:, 0:1],
                    in1=s1h[:, 1:2],
                    op=Alu.add,
                )
                ysq = opool.tile([P, DM], f32, tag="ysq", bufs=1, name="ysq")
                s2 = small.tile([P, 1], f32, tag="s2", name="s2")
                nc.vector.scalar_tensor_tensor(
                    out=ysq,
                    in0=y,
                    scalar=1.0,
                    in1=y,
                    op0=Alu.mult,
                    op1=Alu.mult,
                    accum_out=s2,
                )
                mean = small.tile([P, 1], f32, tag="mean", name="mean")
                nc.vector.tensor_scalar_mul(mean, s1, 1.0 / DM)
                m2 = small.tile([P, 1], f32, tag="m2", name="m2")
                nc.vector.tensor_mul(m2, mean, s1)
                dv = small.tile([P, 1], f32, tag="dv", name="dv")
                nc.vector.tensor_tensor(out=dv, in0=s2, in1=m2, op=Alu.subtract)
                lnv = small.tile([P, 1], f32, tag="lnv", name="lnv")
                nc.scalar.activation(
                    out=lnv, in_=dv, func=Act.Ln, scale=1.0 / (DM - 1)
                )
                sd = small.tile([P, 1], f32, tag="sd", name="sd")
                nc.scalar.activation(out=sd, in_=lnv, func=Act.Exp, scale=0.5)
                nc.vector.tensor_scalar(
                    out=sd, in0=sd, scalar1=EPS, scalar2=None, op0=Alu.add
                )
                ri = small.tile([P, 1], f32, tag="ri", name="ri")
                nc.vector.reciprocal(ri, sd)
                nc.vector.scalar_tensor_tensor(
                    out=y,
                    in0=y,
                    scalar=mean,
                    in1=vrep[:, 0, :],
                    op0=Alu.subtract,
                    op1=Alu.mult,
                )
                nc.scalar.activation(out=y, in_=y, func=Act.Copy, scale=ri)
                yo = opool.tile([P, DM], f32, tag="yo", name="yo")
                nc.vector.tensor_tensor(
                    out=yo, in0=y, in1=vrep[:, 1, :], op=Alu.add
                )
                nc.scalar.dma_start(
                    out=out_d[q * TB : (q + 1) * TB, :], in_=yo
                )

            # ---------------- own-column offset register ----------------
            cofs = const.tile([1, 1], mybir.dt.uint32, name="cofs")
            nc.sync.dma_start(out=cofs, in_=colofs_d[:, :])
            creg = nc.sync.alloc_register("cofs_reg")
            nc.sync.reg_load(creg, cofs[0:1, 0:1])
            colsv = nc.sync.snap(creg, donate=True, min_val=0, max_val=QT - TB)

            # ---------------- main schedule ----------------
            # o_block(q) is processed TWO quarters later: o_dma late in unit
            # (0, q+2) (only the dynamic DMA queue waits on the gather),
            # the reciprocal chain early in unit (1, q+2), and o_main late
            # in unit (1, q+2). o(2) runs in the tail shadowing AG(3);
            # o(3) is the unavoidable tail.
            for q in range(NQ):
                st_box = {}
                feed0 = []
                if q == 0:
                    # V chunks woven into unit (0,0), two per kc-pair step
                    def vc2(i):
                        v_chunk(2 * i)
                        v_chunk(2 * i + 1)

                    feed0 = [
                        (lambda i=i: vc2(i)) for i in range(KT_CH // 2)
                    ]
                elif q >= 2:
                    def f_dma(q=q, st_box=st_box):
                        o_dma(q - 2, st_box)

                    feed0 = [lambda: None] * 6 + [f_dma]
                q_proj(0, q)
                attn_unit(0, q, feed=feed0)
                q_proj(1, q)
                feed1 = []
                if q >= 2:
                    def f_dve(q=q, st_box=st_box):
                        o_dve(q - 2, st_box)

                    def f_main(q=q, st_box=st_box):
                        o_zz(q - 2, st_box)
                        o_fin(q - 2, st_box)

                    feed1 = [lambda: None, f_dve] + [lambda: None] * 3 + [f_main]
                if q == NQ - 1:
                    launch_ag3(0)
                attn_unit(1, q, feed=feed1)
                if q == 0:
                    # remaining xq quarters land during quarter-0 attention
                    # (scalar queue: keeps the sync queue free for staging)
                    for h_ in range(2):
                        nc.scalar.dma_start(
                            out=xq[:, 4 * h_ : 4 * h_ + 4, QT:],
                            in_=xqr[:, 4 * h_ : 4 * h_ + 4, QT:],
                        )
                if q < NQ - 1:
                    launch_ag(q)
                else:
                    launch_ag3(1)
            st2 = {}
            o_dma(NQ - 2, st2)
            o_dve(NQ - 2, st2)
            st3 = {}
            o_dma(NQ - 1, st3, pairs=(0,))
            o_dve(NQ - 1, st3, pairs=(0,))
            o_zz(NQ - 2, st2)
            o_fin(NQ - 2, st2)
            o_zz(NQ - 1, st3, pairs=(0,))
            o_dma(NQ - 1, st3, pairs=(1,))
            o_dve(NQ - 1, st3, pairs=(1,))
            o_zz(NQ - 1, st3, pairs=(1,))
            o_fin(NQ - 1, st3)

    nc.compile()
    _scrub_debug_paths(nc, mybir)
    return nc


def _scrub_debug_paths(nc, mybir):
    """Normalize per-instruction debug info so the serialized module — and
    therefore the neuron compile-cache key — is stable across run dirs."""
    for fn in nc.m.functions:
        stack = list(fn.blocks)
        while stack:
            blk = stack.pop()
            for inst in blk.instructions:
                d = inst.debug
                if d is None:
                    continue
                if d.filename is None and d.ant_traceback is None:
                    continue
                inst.debug = mybir.OpDebugInfo(
                    op_name=d.op_name,
                    tensorizer_id=d.tensorizer_id,
                    filename="kernel.py" if d.filename else None,
                    lineno=d.lineno,
                    bass_funcname=d.bass_funcname,
                    kernel_name=d.kernel_name,
                    ant_traceback=None,
                    ant_layer=d.ant_layer,
                    ant_annotation=d.ant_annotation,
                )
            sub = getattr(blk, "blocks", None)
            if sub:
                stack.extend(sub)
        for alloc in fn.allocations:
            mlocs = getattr(alloc, "memorylocations", None) or []
            for ml in mlocs:
                d = getattr(ml, "ant_debug", None)
                if d is None:
                    continue
                if d.filename is None and d.ant_traceback is None:
                    continue
                ml.ant_debug = mybir.OpDebugInfo(
                    op_name=d.op_name,
                    tensorizer_id=d.tensorizer_id,
                    filename="kernel.py" if d.filename else None,
                    lineno=d.lineno,
                    bass_funcname=d.bass_funcname,
                    kernel_name=d.kernel_name,
                    ant_traceback=None,
                    ant_layer=d.ant_layer,
                    ant_annotation=d.ant_annotation,
                )


def _get_nc():
    global _NC
    if _NC is None:
        _NC = _build_nc()
    return _NC


def _make_in_maps(query, key, value, Wq, bq, Wk, bk, Wv, bv, Wo, bo, gamma, beta):
    qs = np.asarray(query, np.float32)
    ks = np.asarray(key, np.float32)
    vs = np.asarray(value, np.float32)
    FP8 = ml_dtypes.float8_e4m3fn
    # QKV weights are stored x16 in fp8 (values ~U(-0.5,0.5) stay in the
    # normal range); the 1/16 is folded into the bias-add on device.
    wqT = (np.asarray(Wq, np.float32).T * 16.0).astype(FP8)
    wkT = (np.asarray(Wk, np.float32).T * 16.0).astype(FP8)
    wvT = (np.asarray(Wv, np.float32).T * 16.0).astype(FP8)
    woT = (np.asarray(Wo, np.float32).T * 16.0).astype(FP8)
    bq32 = np.asarray(bq, np.float32)
    bk32 = np.asarray(bk, np.float32)
    bv32 = np.asarray(bv, np.float32)
    bo32 = np.asarray(bo, np.float32)
    wo32 = np.asarray(Wo, np.float32)

    # onehot[hb, col]: spread row hb (1/Z of head hb of a pair) onto
    # partitions hb*64..hb*64+63
    onehot = np.zeros((2, P), BF16)
    for hb in range(2):
        onehot[hb, hb * DH : (hb + 1) * DH] = 1.0

    xT = {}
    for b in range(B):
        xT[("q", b)] = np.ascontiguousarray(qs[b].T).astype(FP8)
        xT[("k", b)] = np.ascontiguousarray(ks[b].T).astype(FP8)
        xT[("v", b)] = np.ascontiguousarray(vs[b].T).astype(FP8)

    # V bias folded through the O-projection into the residual: softmax
    # weights sum to 1, so attn with biased V = attn|bv=0 + bv, and
    # (xu + bv) @ Wo.T + bo = xu @ Wo.T + (bv @ Wo.T + bo).
    rbias = bo32 + bv32 @ wo32.T

    in_maps = []
    for core in range(8):
        b, r = divmod(core, 4)
        hsl = slice(r * 2 * P, (r + 1) * 2 * P)  # own 256 dm rows
        vecs = np.stack(
            [np.asarray(gamma, np.float32), np.asarray(beta, np.float32)]
        ).astype(BF16)
        # own output tokens: q*512 + r*128 + i  -> resid row q*128+i
        tok = (
            np.arange(NQ)[:, None] * QT + r * TB + np.arange(TB)[None, :]
        ).reshape(-1)
        in_maps.append(
            {
                "xqT": xT[("q", b)],
                "xkT": xT[("k", b)],
                "xvT": xT[("v", b)],
                "wqT": np.ascontiguousarray(wqT[:, hsl]),
                "wkT": np.ascontiguousarray(wkT[:, hsl]),
                "wvT": np.ascontiguousarray(wvT[:, hsl]),
                "woT": woT,
                "bqp": np.ascontiguousarray(bq32[hsl].reshape(2, P).T),
                "bkp": np.ascontiguousarray(bk32[hsl].reshape(2, P).T),
                "vecs": vecs,
                "onehot": onehot,
                "resid": qs[b][tok] + rbias,
                "colofs": np.array([[r * TB]], np.uint32),
            }
        )
    return in_maps


def _assemble(results):
    out = np.empty((B, S, DM), np.float32)
    for core in range(8):
        b, r = divmod(core, 4)
        res = np.asarray(results[core]["out"], np.float32).reshape(NQ, TB, DM)
        for q in range(NQ):
            out[b, q * QT + r * TB : q * QT + (r + 1) * TB] = res[q]
    return out


def run_sharded(inputs, trace=False, **kwargs):
    """Run on 8 cores; returns (full_output, BassKernelResults)."""
    from concourse.bass_utils import run_bass_kernel_spmd

    nc = _get_nc()
    in_maps = _make_in_maps(
        inputs["query"], inputs["key"], inputs["value"],
        inputs["Wq"], inputs["bq"], inputs["Wk"], inputs["bk"],
        inputs["Wv"], inputs["bv"], inputs["Wo"], inputs["bo"],
        inputs["gamma"], inputs["beta"],
    )
    res = run_bass_kernel_spmd(nc, in_maps, core_ids=list(range(8)), trace=trace, **kwargs)
    return _assemble(res.results), res


def kernel(query, key, value, mask, Wq, bq, Wk, bk, Wv, bv, Wo, bo, gamma, beta):
    out, _ = run_sharded(
        {
            "query": query, "key": key, "value": value,
            "Wq": Wq, "bq": bq, "Wk": Wk, "bk": bk,
            "Wv": Wv, "bv": bv, "Wo": Wo, "bo": bo,
            "gamma": gamma, "beta": beta,
        }
    )
    return out
